# revision 1
# baseline (speedup 1.0000x reference)
"""Bass/Trainium2 kernel for nn_Expert_WNO2d (8-expert gated WaveConv2d mixture).

Math: the reference is linear in x. Every expert passes the fine Haar detail
levels (1..3) through unchanged and only channel-mixes the coarsest (level-4)
approximation + detail coefficients. With gate slots s weighting experts
PERM = (0,1,2,3,4,5,4,5), the output collapses to

    y[b] = G[b] * x[b] + rep8( adj[b] )                      (rep8 = 8x8 block broadcast)
    adj[b] = 0.125 * idwt4( sum_e geff[b,e] * (W_e . c4[b]) ) - (G[b]/64) * s8[b]

where s8 = 8x8 block sums of x, c4 = level-4 Haar coefficients (from s8),
G[b] = sum_s lambda[b,s], geff[b,e] = gate mass routed to expert e.

Sharding: data-parallel over batch B=32 across 8 cores (4 samples/core);
the [6,4,C,C,4,4] expert weights are replicated in bf16 (packed host-side
into the exact SBUF image, scaled by 0.0625 to fold the idwt/broadcast
constants). x streams in 1MB sub-tiles with partial block-sum reduces;
weights stream per-band so matmuls start before the full load; the final
fused pass streams per sub-tile (DVE + GpSimd) so y DMA-out overlaps.
"""

import numpy as np

import concourse.bacc as bacc
import concourse.mybir as mybir
import concourse.tile as tile

N_CORES = 8
B, C, S = 32, 64, 64
BL = B // N_CORES          # samples per core = 4
NE = 6                     # live experts
NCH = 4                    # x sub-tiles per row-tile
f32 = mybir.dt.float32
bf16 = mybir.dt.bfloat16
ALU = mybir.AluOpType


def _build_nc():
    nc = bacc.Bacc()
    xw = nc.declare_dram_parameter("xw", [2, 128, 4096], f32, isOutput=False)
    wt = nc.declare_dram_parameter("wt", [4, 128, 3072], bf16, isOutput=False)
    gt = nc.declare_dram_parameter("gt", [2, 128, 8], f32, isOutput=False)
    yw = nc.declare_dram_parameter("yw", [2, 128, 4096], f32, isOutput=True)

    with tile.TileContext(nc) as tc:
        with (
            tc.tile_pool(name="xp", bufs=8) as xp,
            tc.tile_pool(name="yp", bufs=8) as yp,
            tc.tile_pool(name="wp", bufs=4) as wp,
            tc.tile_pool(name="sp", bufs=2) as sp,
            tc.tile_pool(name="cp", bufs=3) as cp,
            tc.tile_pool(name="tp", bufs=8) as ttp,
            tc.tile_pool(name="ps", bufs=4, space="PSUM") as psp,
        ):
            gt_s, xs = [], [[], []]
            for rt in range(2):
                g = sp.tile([128, 8], f32, tag="gt", name=f"g{rt}")
                nc.sync.dma_start(out=g[:, :], in_=gt[rt, :, :])
                gt_s.append(g)

            wt_b = []
            for band in range(4):
                w = wp.tile([128, 3072], bf16, tag="wt", name=f"w{band}")
                wt_b.append(w)

            # interleave x sub-tile and weight-band DMA issue; x leads
            order = [("x", 0, 0), ("x", 0, 1), ("x", 0, 2), ("x", 0, 3),
                     ("x", 1, 0), ("x", 1, 1), ("x", 1, 2), ("x", 1, 3),
                     ("w", 0, 0), ("w", 1, 0), ("w", 2, 0), ("w", 3, 0)]
            for kind, a, c in order:
                if kind == "x":
                    xt = xp.tile([128, 1024], f32, tag="xs", name=f"x{a}{c}")
                    nc.sync.dma_start(out=xt[:, :], in_=xw[a, :, 1024 * c:1024 * (c + 1)])
                    xs[a].append(xt)
                else:
                    nc.sync.dma_start(out=wt_b[a][:, :], in_=wt[a, :, :])

            cc = cp.tile([128, 768], bf16, tag="cc", name="cc")
            coef, s8 = [], []
            for rt in range(2):
                # 8x8 block sums, streamed per sub-tile (w-dir), then h-dir
                r1 = sp.tile([128, 512], f32, tag="r1", name=f"r1{rt}")
                for c in range(NCH):
                    nc.vector.tensor_reduce(
                        out=r1[:, 128 * c:128 * (c + 1)].rearrange("p (h v) -> p h v", h=16),
                        in_=xs[rt][c][:, :].rearrange("p (h v w) -> p h v w", h=16, v=8, w=8),
                        axis=mybir.AxisListType.X, op=ALU.add,
                    )
                s8t = sp.tile([128, 64], f32, tag="s8", name=f"s8{rt}")
                nc.vector.tensor_reduce(
                    out=s8t[:, :].rearrange("p (u v) -> p u v", u=8),
                    in_=r1[:, :].rearrange("p (u dh v) -> p u v dh", u=8, dh=8, v=8),
                    axis=mybir.AxisListType.X, op=ALU.add,
                )
                s8.append(s8t)

                # level-4 Haar analysis on 0.0625*s8 (scale folds ll3 + one dwt level)
                sc = sp.tile([128, 64], f32, tag="sc", name=f"sc{rt}")
                nc.vector.tensor_scalar(out=sc[:, :], in0=s8t[:, :],
                                        scalar1=0.0625, scalar2=None, op0=ALU.mult)
                # merged quad combines: {t1,t2} = even+odd, {t3,t4} = even-odd
                # even = {a00,a10}: offsets {0,8}; odd = {a01,a11}: offsets {1,9}
                ev = sc[:, 0:64].rearrange("p (x i y j) -> p i j x y",
                                           x=4, i=2, y=4, j=2)[:, :, 0]
                od = sc[:, 0:64].rearrange("p (x i y j) -> p i j x y",
                                           x=4, i=2, y=4, j=2)[:, :, 1]
                tt = ttp.tile([128, 64], f32, tag="tt", name=f"tt{rt}")
                t2v = lambda o: tt[:, 32 * o:32 * (o + 1)].rearrange(
                    "p (g x y) -> p g x y", g=2, x=4, y=4)
                nc.vector.tensor_add(t2v(0), ev, od)   # t1(a00+a01), t2(a10+a11)
                nc.vector.tensor_sub(t2v(1), ev, od)   # t3, t4
                cf = sp.tile([128, 64], f32, tag="coef", name=f"cf{rt}")
                pick = lambda t, o: t[:, :].rearrange(
                    "p (g h m) -> p h g m", g=2, h=2, m=16)[:, o]
                nc.vector.tensor_add(pick(cf, 0), pick(tt, 0), pick(tt, 1))  # ll, hl
                nc.vector.tensor_sub(pick(cf, 1), pick(tt, 0), pick(tt, 1))  # lh, hh
                coef.append(cf)

            # gate-scaled channel-transposed coefficients:
            # cc[el*64+i, ch*256 + b*64 + bm], one op per (rt, bh, el):
            # out spans the 3 ch blocks; in0 broadcasts cf over ch; the gate
            # operand walks gt cols 1+el, 3+el, 5+el (stride 2) per ch block.
            for rt in range(2):
                cf = coef[rt]
                for bh in range(2):
                    b = rt * 2 + bh
                    for el in range(2):
                        nc.vector.tensor_tensor(
                            out=cc[el * 64:(el + 1) * 64, :]
                                .rearrange("p (ch bb m) -> p ch bb m", ch=3, bb=4, m=64)[:, :, b],
                            in0=cf[bh * 64:(bh + 1) * 64, :]
                                .rearrange("p (o m) -> p o m", o=1)
                                .broadcast_to([64, 3, 64]),
                            in1=gt_s[rt][bh * 64:(bh + 1) * 64, 1 + el:6 + el:2]
                                .rearrange("p (c o) -> p c o", c=3, o=1)
                                .broadcast_to([64, 3, 64]),
                            op=ALU.mult,
                        )

            # per-mode channel mixing, gate-combined via K=(e,i) accumulation
            pb = [psp.tile([64, 64], f32, tag="pb", name=f"pb{i}") for i in range(4)]
            for band in range(4):
                for mode in range(16):
                    for ch in range(3):
                        nc.tensor.matmul(
                            out=pb[band][:, mode * 4:(mode + 1) * 4],
                            lhsT=wt_b[band][:, (mode * 3 + ch) * 64:(mode * 3 + ch + 1) * 64],
                            rhs=cc[:, ch * 256 + band * 16 + mode:ch * 256 + band * 16 + mode + 193:64],
                            start=(ch == 0), stop=(ch == 2),
                        )

            # level-4 Haar synthesis (scale folded into weights) scattered per-sample
            sb1 = ttp.tile([64, 64], f32, tag="sb1")
            sb3 = ttp.tile([64, 64], f32, tag="sb3")
            nc.vector.tensor_copy(sb1[:, :], pb[1][:, :])
            nc.vector.tensor_copy(sb3[:, :], pb[3][:, :])
            u13 = ttp.tile([64, 128], f32, tag="u13")
            u24 = ttp.tile([64, 128], f32, tag="u24")
            nc.vector.tensor_add(u13[:, 0:64], pb[0][:, :], sb1[:, :])
            nc.vector.tensor_add(u24[:, 0:64], pb[2][:, :], sb3[:, :])
            nc.vector.tensor_sub(u13[:, 64:128], pb[0][:, :], sb1[:, :])
            nc.vector.tensor_sub(u24[:, 64:128], pb[2][:, :], sb3[:, :])

            adj_hs = []
            for rt in range(2):
                at = sp.tile([128, 64], f32, tag="adjT", name=f"at{rt}")
                for bh in range(2):
                    b = rt * 2 + bh
                    ov = at[bh * 64:(bh + 1) * 64, :].rearrange(
                        "p (x di y dj) -> p dj di x y", x=4, di=2, y=4, dj=2)
                    sv = lambda t: t[:, :].rearrange(
                        "p (k x y bb) -> p bb k x y", k=2, x=4, y=4, bb=4)[:, b]
                    nc.vector.tensor_add(ov[:, 0], sv(u13), sv(u24))
                    nc.vector.tensor_sub(ov[:, 1], sv(u13), sv(u24))
                # adjF = adjT + (-G/64) * s8   (gt col 7 = -G/64)
                adjF = sp.tile([128, 64], f32, tag="adjF", name=f"af{rt}")
                nc.vector.scalar_tensor_tensor(
                    out=adjF[:, :], in0=s8[rt][:, :], scalar=gt_s[rt][:, 7:8],
                    in1=at[:, :], op0=ALU.mult, op1=ALU.add,
                )
                # expand over h-rep: adj_h[p, u*64 + dh*8 + v] = adjF[p, u*8+v]
                adj_h = sp.tile([128, 512], f32, tag="adjh", name=f"ah{rt}")
                nc.vector.tensor_copy(
                    out=adj_h[:, :].rearrange("p (u dh v) -> p u dh v", u=8, dh=8, v=8),
                    in_=adjF[:, :].rearrange("p (u o v) -> p u o v", u=8, o=1, v=8)
                        .broadcast_to([128, 8, 8, 8]),
                )
                adj_hs.append(adj_h)

            # y = G*x + rep8(adjF), one fused DVE pass per sub-tile, stores stream out
            for rt in range(2):
                for c in range(NCH):
                    ys = yp.tile([128, 1024], f32, tag="ys", name=f"y{rt}{c}")
                    nc.vector.scalar_tensor_tensor(
                        out=ys[:, :].rearrange("p (hv w) -> p hv w", w=8),
                        in0=xs[rt][c][:, :].rearrange("p (hv w) -> p hv w", w=8),
                        scalar=gt_s[rt][:, 0:1],
                        in1=adj_hs[rt][:, 128 * c:128 * (c + 1)]
                            .rearrange("p (hv o) -> p hv o", o=1)
                            .broadcast_to([128, 128, 8]),
                        op0=ALU.mult, op1=ALU.add,
                    )
                    nc.sync.dma_start(out=yw[rt, :, 1024 * c:1024 * (c + 1)], in_=ys[:, :])
    nc.compile()
    return nc


_NC = None


def _get_nc():
    global _NC
    if _NC is None:
        _NC = _build_nc()
    return _NC


def _pack_weights(WL, WH):
    # Wall[band, e, i, o, x, y]; band 0 = WL, bands 1..3 = WH[:, k-1]
    Wall = np.empty((4, NE, C, C, 4, 4), np.float32)
    Wall[0] = WL[:NE]
    for k in range(3):
        Wall[k + 1] = WH[:NE, k]
    Wall *= 0.0625  # folds idwt 0.5 and rep8 0.125 scales
    # wt[band][el*64+i, ((x*4+y)*3 + ch)*64 + o]
    W6 = Wall.reshape(4, 3, 2, C, C, 4, 4)            # band, ch, el, i, o, x, y
    T = W6.transpose(0, 2, 3, 5, 6, 1, 4)             # band, el, i, x, y, ch, o
    import ml_dtypes
    return np.ascontiguousarray(T.reshape(4, 128, 3072)).astype(ml_dtypes.bfloat16)


def _pack_gates(lambda_):
    lam = lambda_.reshape(B, 8).astype(np.float32)
    G = lam.sum(1)
    geff = lam[:, :6].copy()
    geff[:, 4] += lam[:, 6]
    geff[:, 5] += lam[:, 7]
    gt = np.zeros((B, 8), np.float32)
    gt[:, 0] = G
    gt[:, 1:7] = geff
    gt[:, 7] = -G / 64.0
    return gt


def kernel(x, lambda_, WL, WH):
    from concourse.bass_utils import run_bass_kernel_spmd

    nc = _get_nc()
    wt = _pack_weights(np.asarray(WL, np.float32), np.asarray(WH, np.float32))
    gt = _pack_gates(np.asarray(lambda_, np.float32))
    x = np.ascontiguousarray(np.asarray(x, np.float32))

    in_maps = []
    for k in range(N_CORES):
        xl = x[k * BL:(k + 1) * BL].reshape(2, 128, 4096)
        gl = np.repeat(gt[k * BL:(k + 1) * BL], C, axis=0).reshape(2, 128, 8)
        in_maps.append({"xw": np.ascontiguousarray(xl),
                        "wt": wt,
                        "gt": np.ascontiguousarray(gl)})

    res = run_bass_kernel_spmd(nc, in_maps, list(range(N_CORES)))
    out = np.empty((B, C, S, S), np.float32)
    for k in range(N_CORES):
        out[k * BL:(k + 1) * BL] = res.results[k]["yw"].reshape(BL, C, S, S)
    return out



# revision 8
# speedup vs baseline: 1.1346x; 1.1346x over previous
"""Bass/Trainium2 kernel for nn_Expert_WNO2d (8-expert gated WaveConv2d mixture).

Math: the reference is linear in x. Every expert passes the fine Haar detail
levels (1..3) through unchanged and only channel-mixes the coarsest (level-4)
approximation + detail coefficients. With gate slots s weighting experts
PERM = (0,1,2,3,4,5,4,5), the output collapses to

    y[b] = G[b] * x[b] + rep8( adj[b] )                      (rep8 = 8x8 block broadcast)
    adj[b] = 0.125 * idwt4( sum_e geff[b,e] * (W_e . c4[b]) ) - (G[b]/64) * s8[b]

where s8 = 8x8 block sums of x, c4 = level-4 Haar coefficients (from s8),
G[b] = sum_s lambda[b,s], geff[b,e] = gate mass routed to expert e.

Sharding: data-parallel over batch B=32 across 8 cores (4 samples/core).
The kernel is memory-bound, so all bulk traffic is narrow: x streams in as
bf16 (host-cast), y streams out as bf16 (host-upcast), and the replicated
[6,4,C,C,4,4] expert weights are fp8 e4m3 (scaled 2^16 to dodge fp8
subnormals; undone on the tiny `at` tensor after PSUM). Stationary matmul
operands are packed 128 columns wide (two modes side by side; only the
diagonal PSUM quadrants are read) so the compiler's fast-weight-load path
engages. 8x8 block sums are one multi-axis reduce per x chunk; the final
y = G*x + rep8(adj) pass is split across Vector and GpSimd with the adj
broadcast folded into the op, and y stores issue from the Activation
engine's DMA queue while input DMA owns the Sync queue.
"""

import numpy as np

import concourse.bacc as bacc
import concourse.mybir as mybir
import concourse.tile as tile

N_CORES = 8
B, C, S = 32, 64, 64
BL = B // N_CORES          # samples per core = 4
f32 = mybir.dt.float32
bf16 = mybir.dt.bfloat16
f8 = mybir.dt.float8e4
ALU = mybir.AluOpType
WSCALE = 2.0 ** 16         # fp8 weight scale (weights ~1e-5 underflow fp8 otherwise)


def _build_nc():
    nc = bacc.Bacc()
    xw = nc.declare_dram_parameter("xw", [2, 128, 4096], bf16, isOutput=False)
    wt = nc.declare_dram_parameter("wt", [4, 128, 3072], f8, isOutput=False)
    gt = nc.declare_dram_parameter("gt", [128, 16], f32, isOutput=False)
    yw = nc.declare_dram_parameter("yw", [2, 128, 4096], bf16, isOutput=True)

    with tile.TileContext(nc) as tc:
        with (
            tc.tile_pool(name="xp", bufs=4) as xp,
            tc.tile_pool(name="yp", bufs=8) as yp,
            tc.tile_pool(name="wp", bufs=4) as wp,
            tc.tile_pool(name="sp", bufs=2) as sp,
            tc.tile_pool(name="ps", bufs=4, space="PSUM") as psp,
        ):
            # ---- input DMA: gt, x chunks, weight bands (Sync queue, in order)
            gt_s = sp.tile([128, 16], f32, tag="gt", name="gt")
            nc.sync.dma_start(out=gt_s[:, :], in_=gt[:, :])

            xc = []
            for q in range(4):                       # chunk q: rt=q>>1, half=q&1
                xt = xp.tile([128, 2048], bf16, tag="xs", name=f"x{q}")
                nc.sync.dma_start(
                    out=xt[:, :], in_=xw[q >> 1, :, 2048 * (q & 1):2048 * (q & 1) + 2048])
                xc.append(xt)

            wt_b = []
            for band in range(4):
                w = wp.tile([128, 3072], f8, tag="wt", name=f"w{band}")
                nc.sync.dma_start(out=w[:, :], in_=wt[band, :, :])
                wt_b.append(w)

            # ---- 8x8 block sums per chunk: one multi-axis reduce each (DVE)
            s8 = []
            for rt in range(2):
                s8t = sp.tile([128, 64], f32, tag="s8", name=f"s8{rt}")
                for h in range(2):
                    nc.vector.tensor_reduce(
                        out=s8t[:, 32 * h:32 * (h + 1)].rearrange("p (u v) -> p u v", u=4),
                        in_=xc[rt * 2 + h][:, :].rearrange(
                            "p (u dh v w) -> p u v dh w", u=4, dh=8, v=8, w=8),
                        axis=mybir.AxisListType.XY, op=ALU.add,
                    )
                s8.append(s8t)

            # ---- level-4 Haar analysis + gate-scaled fp8 coefficients
            # cf col = band*16 + pr*2 + par   (band order ll,lh,hl,hh; mode=x*4+y)
            # cc[el*64+i, ch*256 + band*64 + pr*8 + par*4 + b]
            cc = sp.tile([128, 768], f8, tag="cc", name="cc")
            coef = []
            for rt in range(2):
                sc = sp.tile([128, 64], f32, tag="sc", name=f"sc{rt}")
                nc.vector.tensor_scalar(out=sc[:, :], in0=s8[rt][:, :],
                                        scalar1=0.0625, scalar2=None, op0=ALU.mult)
                ev = sc[:, 0:64].rearrange("p (x i y j) -> p i j x y",
                                           x=4, i=2, y=4, j=2)[:, :, 0]
                od = sc[:, 0:64].rearrange("p (x i y j) -> p i j x y",
                                           x=4, i=2, y=4, j=2)[:, :, 1]
                tt = sp.tile([128, 64], f32, tag="tt", name=f"tt{rt}")
                t2v = lambda o: tt[:, 32 * o:32 * (o + 1)].rearrange(
                    "p (g x y) -> p g x y", g=2, x=4, y=4)
                nc.vector.tensor_add(t2v(0), ev, od)
                nc.vector.tensor_sub(t2v(1), ev, od)
                cf = sp.tile([128, 64], f32, tag="coef", name=f"cf{rt}")
                pick = lambda t, o: t[:, :].rearrange(
                    "p (g h m) -> p h g m", g=2, h=2, m=16)[:, o]
                nc.vector.tensor_add(pick(cf, 0), pick(tt, 0), pick(tt, 1))  # ll, hl
                nc.vector.tensor_sub(pick(cf, 1), pick(tt, 0), pick(tt, 1))  # lh, hh
                coef.append(cf)

                for bh in range(2):
                    b = rt * 2 + bh
                    for el in range(2):
                        nc.vector.tensor_tensor(
                            out=cc[el * 64:(el + 1) * 64, b::4].rearrange(
                                "p (ch band m) -> p ch band m", ch=3, band=4, m=16),
                            in0=cf[bh * 64:(bh + 1) * 64, :].rearrange(
                                "p (o band m) -> p o band m",
                                o=1, band=4, m=16).broadcast_to([64, 3, 4, 16]),
                            in1=gt_s[bh * 64:(bh + 1) * 64, 8 * rt + 1 + el:8 * rt + 6 + el:2]
                                .rearrange("p (ch u v) -> p ch u v", ch=3, u=1, v=1)
                                .broadcast_to([64, 3, 4, 16]),
                            op=ALU.mult,
                        )

            # ---- per-mode channel mixing: 128-col stationaries (mode pairs),
            # gate-combined via K=(el,i), ch accumulated in PSUM. Only the
            # par-diagonal quadrants of pb are valid.
            pb = [psp.tile([128, 64], f32, tag="pb", name=f"pb{i}") for i in range(4)]
            for band in range(4):
                for pr in range(8):
                    for ch in range(3):
                        nc.tensor.matmul(
                            out=pb[band][:, pr * 8:(pr + 1) * 8],
                            lhsT=wt_b[band][:, (pr * 3 + ch) * 128:(pr * 3 + ch + 1) * 128],
                            rhs=cc[:, ch * 256 + band * 64 + pr * 8:ch * 256 + band * 64 + pr * 8 + 8],
                            start=(ch == 0), stop=(ch == 2),
                        )

            # ---- level-4 Haar synthesis from PSUM quadrants
            # SD[:, di*64 + pr*8 + par*4 + b]: di=0 -> ll+lh, di=1 -> ll-lh
            SD = sp.tile([128, 128], f32, tag="SD", name="SD")
            TU = sp.tile([128, 128], f32, tag="TU", name="TU")
            sb1 = sp.tile([128, 64], f32, tag="sb1", name="sb1")
            sb3 = sp.tile([128, 64], f32, tag="sb3", name="sb3")
            nc.vector.tensor_copy(sb1[:, :], pb[1][:, :])
            nc.vector.tensor_copy(sb3[:, :], pb[3][:, :])
            nc.vector.tensor_add(SD[:, 0:64], pb[0][:, :], sb1[:, :])
            nc.vector.tensor_sub(SD[:, 64:128], pb[0][:, :], sb1[:, :])
            nc.vector.tensor_add(TU[:, 0:64], pb[2][:, :], sb3[:, :])
            nc.vector.tensor_sub(TU[:, 64:128], pb[2][:, :], sb3[:, :])

            adjF = []
            for rt in range(2):
                at = sp.tile([128, 64], f32, tag="at", name=f"at{rt}")
                for bh in range(2):
                    b = rt * 2 + bh
                    ov = at[bh * 64:(bh + 1) * 64, :].rearrange(
                        "p (x di yy par dj) -> p x di yy par dj",
                        x=4, di=2, yy=2, par=2, dj=2)
                    sv = lambda t, par: t[par * 64:(par + 1) * 64, :].rearrange(
                        "p (di x yy pq bb) -> p x di yy pq bb",
                        di=2, x=4, yy=2, pq=2, bb=4)[:, :, :, :, par, b]
                    for par in range(2):
                        nc.vector.tensor_add(ov[:, :, :, :, par, 0], sv(SD, par), sv(TU, par))
                        nc.vector.tensor_sub(ov[:, :, :, :, par, 1], sv(SD, par), sv(TU, par))
                # undo fp8 weight scale on the tiny tensor
                nc.vector.tensor_scalar(out=at[:, :], in0=at[:, :],
                                        scalar1=1.0 / WSCALE, scalar2=None, op0=ALU.mult)
                # adjF = at + (-G/64) * s8   (gt col 7 = -G/64)
                af = sp.tile([128, 64], f32, tag="adjF", name=f"af{rt}")
                nc.vector.scalar_tensor_tensor(
                    out=af[:, :], in0=s8[rt][:, :], scalar=gt_s[:, 8 * rt + 7:8 * rt + 8],
                    in1=at[:, :], op0=ALU.mult, op1=ALU.add,
                )
                # expand over h-rep: adj_h[p, u*64 + dh*8 + v] = adjF[p, u*8+v]
                ah = sp.tile([128, 512], f32, tag="adjh", name=f"ah{rt}")
                nc.vector.tensor_copy(
                    out=ah[:, :].rearrange("p (u dh v) -> p u dh v", u=8, dh=8, v=8),
                    in_=af[:, :].rearrange("p (u o v) -> p u o v", u=8, o=1, v=8)
                        .broadcast_to([128, 8, 8, 8]),
                )
                adjF.append(ah)

            # ---- y = G*x + rep8(adjF): fused pass per 1024-col piece,
            # split Vector/GpSimd, stores on the Activation DMA queue
            for rt in range(2):
                for p in range(4):
                    s = rt * 4 + p
                    ys = yp.tile([128, 1024], bf16, tag="ys", name=f"y{s}")
                    xin = xc[rt * 2 + (p >> 1)][:, 1024 * (p & 1):1024 * (p & 1) + 1024]
                    adj_in = adjF[rt][:, 128 * p:128 * (p + 1)] \
                        .rearrange("p (hv o) -> p hv o", o=1).broadcast_to([128, 128, 8])
                    if p == 3:
                        # GpSimd lacks scalar-ptr STT: gate-mult then adj-add
                        nc.gpsimd.tensor_tensor(
                            out=ys[:, :].rearrange("p (hv w) -> p hv w", w=8),
                            in0=xin.rearrange("p (hv w) -> p hv w", w=8),
                            in1=gt_s[:, 8 * rt:8 * rt + 1]
                                .rearrange("p (a b) -> p a b", a=1, b=1)
                                .broadcast_to([128, 128, 8]),
                            op=ALU.mult,
                        )
                        nc.gpsimd.tensor_tensor(
                            out=ys[:, :].rearrange("p (hv w) -> p hv w", w=8),
                            in0=ys[:, :].rearrange("p (hv w) -> p hv w", w=8),
                            in1=adj_in, op=ALU.add,
                        )
                    else:
                        nc.vector.scalar_tensor_tensor(
                            out=ys[:, :].rearrange("p (hv w) -> p hv w", w=8),
                            in0=xin.rearrange("p (hv w) -> p hv w", w=8),
                            scalar=gt_s[:, 8 * rt:8 * rt + 1],
                            in1=adj_in, op0=ALU.mult, op1=ALU.add,
                        )
                    nc.scalar.dma_start(out=yw[rt, :, 1024 * p:1024 * (p + 1)], in_=ys[:, :])
    nc.compile()
    return nc


_NC = None


def _get_nc():
    global _NC
    if _NC is None:
        _NC = _build_nc()
    return _NC


def _pack_weights(WL, WH):
    import ml_dtypes
    NE = 6
    # Wall[band, e, i, o, x, y]; band 0 = WL, bands 1..3 = WH[:, k-1]; e = ch*2+el
    Wall = np.empty((4, NE, C, C, 4, 4), np.float32)
    Wall[0] = WL[:NE]
    for k in range(3):
        Wall[k + 1] = WH[:NE, k]
    Wall *= 0.0625 * WSCALE            # idwt/rep8 scales + fp8 range scale
    W7 = Wall.reshape(4, 3, 2, C, C, 8, 2)            # band, ch, el, i, o, pr, par
    T = W7.transpose(0, 2, 3, 5, 1, 6, 4)             # band, el, i, pr, ch, par, o
    return np.ascontiguousarray(T.reshape(4, 128, 3072)).astype(ml_dtypes.float8_e4m3fn)


def _pack_gates(lambda_):
    lam = lambda_.reshape(B, 8).astype(np.float32)
    G = lam.sum(1)
    geff = lam[:, :6].copy()
    geff[:, 4] += lam[:, 6]
    geff[:, 5] += lam[:, 7]
    gt = np.zeros((B, 8), np.float32)
    gt[:, 0] = G
    gt[:, 1:7] = geff
    gt[:, 7] = -G / 64.0
    return gt


def _build_in_maps(x, lambda_, WL, WH):
    import ml_dtypes
    wt = _pack_weights(np.asarray(WL, np.float32), np.asarray(WH, np.float32))
    gt = _pack_gates(np.asarray(lambda_, np.float32))
    xb = np.asarray(x, np.float32).astype(ml_dtypes.bfloat16)

    in_maps = []
    for k in range(N_CORES):
        xl = np.ascontiguousarray(xb[k * BL:(k + 1) * BL].reshape(2, 128, 4096))
        g4 = gt[k * BL:(k + 1) * BL]                  # [4, 8], b = rt*2+bh
        gl = np.broadcast_to(
            g4.reshape(2, 2, 1, 8).transpose(1, 2, 0, 3), (2, 64, 2, 8))
        gl = np.ascontiguousarray(gl.reshape(128, 16), dtype=np.float32)
        in_maps.append({"xw": xl, "wt": wt, "gt": gl})
    return in_maps


def kernel(x, lambda_, WL, WH):
    from concourse.bass_utils import run_bass_kernel_spmd

    nc = _get_nc()
    in_maps = _build_in_maps(x, lambda_, WL, WH)
    res = run_bass_kernel_spmd(nc, in_maps, list(range(N_CORES)))
    out = np.empty((B, C, S, S), np.float32)
    for k in range(N_CORES):
        out[k * BL:(k + 1) * BL] = (
            res.results[k]["yw"].astype(np.float32).reshape(BL, C, S, S))
    return out


# revision 12
# speedup vs baseline: 1.2965x; 1.1427x over previous
"""Bass/Trainium2 kernel for nn_Expert_WNO2d (8-expert gated WaveConv2d mixture).

Math: the reference is linear in x. Every expert passes the fine Haar detail
levels (1..3) through unchanged and only channel-mixes the coarsest (level-4)
approximation + detail coefficients. With gate slots s weighting experts
PERM = (0,1,2,3,4,5,4,5), the output collapses to

    y[b] = G[b] * x[b] + rep8( adj[b] )                      (rep8 = 8x8 block broadcast)
    adj[b] = 0.125 * idwt4( sum_e geff[b,e] * (W_e . c4[b]) ) - (G[b]/64) * s8[b]

where s8 = 8x8 block sums of x, c4 = level-4 Haar coefficients (from s8),
G[b] = sum_s lambda[b,s], geff[b,e] = gate mass routed to expert e.

Sharding: data-parallel over batch B=32 across 8 cores (4 samples/core).
Memory-bound, so all bulk traffic is narrow: x in bf16, y out bf16, expert
weights fp8 e4m3 (scaled 2^16 against fp8 subnormals; descaled on the tiny
adj tensor). Matmul stationaries are 128 columns wide (two modes packed;
only par-diagonal PSUM quadrants are read) so fast-weight-load engages, and
the mixing matmuls pipeline per band against the weight DMA stream. The
final y = G*x + rep8(adj) pass is spread over Vector (fused STT), GpSimd
(mult early under the input stream + add late), and TensorE (diag(G) @ x +
I @ adj_bcast into PSUM, Activation copies PSUM->bf16). Input DMA owns the
Sync queue (x before w, single-packet to cut completion-semaphore lag);
y stores follow on Sync; diag/identity constants load on the ACT queue.
"""

import numpy as np

import concourse.bacc as bacc
import concourse.mybir as mybir
import concourse.tile as tile

N_CORES = 8
B, C, S = 32, 64, 64
BL = B // N_CORES          # samples per core = 4
f32 = mybir.dt.float32
bf16 = mybir.dt.bfloat16
f8 = mybir.dt.float8e4
ALU = mybir.AluOpType
WSCALE = 2.0 ** 16         # fp8 weight scale (weights ~1e-5 underflow fp8 otherwise)

PIECE_V = (0, 1)           # Vector fused STT
PIECE_G = (3, 7)           # GpSimd mult (early) + add (late)
PIECE_PE = (2, 4, 5, 6)    # TensorE diag-matmul + ACT psum->bf16 copy


def _build_nc():
    nc = bacc.Bacc()
    xw = nc.declare_dram_parameter("xw", [2, 128, 4096], bf16, isOutput=False)
    wt = nc.declare_dram_parameter("wt", [4, 128, 3072], f8, isOutput=False)
    gt = nc.declare_dram_parameter("gt", [128, 16], f32, isOutput=False)
    dg = nc.declare_dram_parameter("dg", [2, 128, 128], bf16, isOutput=False)
    iw = nc.declare_dram_parameter("iw", [128, 128], bf16, isOutput=False)
    yw = nc.declare_dram_parameter("yw", [2, 128, 4096], bf16, isOutput=True)

    with tile.TileContext(nc) as tc:
        with (
            tc.tile_pool(name="xp", bufs=4) as xp,
            tc.tile_pool(name="yp", bufs=8) as yp,
            tc.tile_pool(name="wp", bufs=4) as wp,
            tc.tile_pool(name="sp", bufs=2) as sp,
            tc.tile_pool(name="ps", bufs=4, space="PSUM") as psp,
            tc.tile_pool(name="py", bufs=2, space="PSUM") as psy,
        ):
            # ---- input DMA on Sync: gt, x chunks (a=3072, b=1024 cols), w bands
            gt_s = sp.tile([128, 16], f32, tag="gt", name="gt")
            nc.sync.dma_start(out=gt_s[:, :], in_=gt[:, :])
            dg_s = sp.tile([128, 256], bf16, tag="dg", name="dg")
            nc.scalar.dma_start(out=dg_s[:, 0:128], in_=dg[0, :, :])
            nc.scalar.dma_start(out=dg_s[:, 128:256], in_=dg[1, :, :])
            iw_s = sp.tile([128, 128], bf16, tag="iw", name="iw")
            nc.scalar.dma_start(out=iw_s[:, :], in_=iw[:, :])

            CHW = (3072, 1024)                       # chunk widths per rt
            xc = []                                  # xc[rt*2+h]
            for rt in range(2):
                for h in range(2):
                    wdt = CHW[h]
                    off = 3072 * h
                    xt = xp.tile([128, wdt], bf16, tag=f"xs{h}", name=f"x{rt}{h}")
                    nc.sync.dma_start(out=xt[:, :], in_=xw[rt, :, off:off + wdt],
                                      single_packet=True)
                    xc.append(xt)

            wt_b = []
            for band in range(4):
                w = wp.tile([128, 3072], f8, tag="wt", name=f"w{band}")
                nc.sync.dma_start(out=w[:, :], in_=wt[band, :, :], single_packet=True)
                wt_b.append(w)

            # ---- 8x8 block sums per chunk (multi-axis reduce, all-bf16 for 2x)
            s8 = [sp.tile([128, 64], bf16, tag="s8", name=f"s8{rt}") for rt in range(2)]
            with nc.allow_low_precision("s8 block sums in bf16; rel err ~0.5% of a "
                                        "small additive term, within 2e-2 gate"):
                for rt in range(2):
                    for h in range(2):
                        u = (6, 2)[h]
                        nc.vector.tensor_reduce(
                            out=s8[rt][:, 48 * h:48 * h + 8 * u]
                                .rearrange("p (u v) -> p u v", u=u),
                            in_=xc[rt * 2 + h][:, :].rearrange(
                                "p (u dh v w) -> p u v dh w", u=u, dh=8, v=8, w=8),
                            axis=mybir.AxisListType.XY, op=ALU.add,
                        )

            # ---- GpSimd early: gate-mult halves of its final pieces (under stream)
            g_ys = {}
            for s in PIECE_G:
                rt, p = s >> 2, s & 3
                ys = yp.tile([128, 1024], bf16, tag="ys", name=f"y{s}")
                g_ys[s] = ys
                xin = xc[rt * 2 + (1 if p == 3 else 0)][:, (1024 * p if p < 3 else 0):][:, 0:1024]
                nc.gpsimd.tensor_tensor(
                    out=ys[:, :].rearrange("p (hv w) -> p hv w", w=8),
                    in0=xin.rearrange("p (hv w) -> p hv w", w=8),
                    in1=gt_s[:, 8 * rt:8 * rt + 1]
                        .rearrange("p (a b) -> p a b", a=1, b=1)
                        .broadcast_to([128, 128, 8]),
                    op=ALU.mult,
                )

            # ---- level-4 Haar analysis + gate-scaled fp8 coefficients
            # cf col = band*16 + m (band order ll,lh,hl,hh; m = x*4+y = pr*2+par)
            # cc[el*64+i, b*192 + ch*64 + band*16 + m]   (contiguous per (b,el))
            cc = sp.tile([128, 768], f8, tag="cc", name="cc")
            for rt in range(2):
                sc = sp.tile([128, 64], f32, tag="sc", name=f"sc{rt}")
                nc.vector.tensor_scalar(out=sc[:, :], in0=s8[rt][:, :],
                                        scalar1=0.0625, scalar2=None, op0=ALU.mult)
                ev = sc[:, 0:64].rearrange("p (x i y j) -> p i j x y",
                                           x=4, i=2, y=4, j=2)[:, :, 0]
                od = sc[:, 0:64].rearrange("p (x i y j) -> p i j x y",
                                           x=4, i=2, y=4, j=2)[:, :, 1]
                tt = sp.tile([128, 64], f32, tag="tt", name=f"tt{rt}")
                t2v = lambda o: tt[:, 32 * o:32 * (o + 1)].rearrange(
                    "p (g x y) -> p g x y", g=2, x=4, y=4)
                nc.vector.tensor_add(t2v(0), ev, od)
                nc.vector.tensor_sub(t2v(1), ev, od)
                cf = sp.tile([128, 64], f32, tag="coef", name=f"cf{rt}")
                pick = lambda t, o: t[:, :].rearrange(
                    "p (g h m) -> p h g m", g=2, h=2, m=16)[:, o]
                nc.vector.tensor_add(pick(cf, 0), pick(tt, 0), pick(tt, 1))  # ll, hl
                nc.vector.tensor_sub(pick(cf, 1), pick(tt, 0), pick(tt, 1))  # lh, hh

                for bh in range(2):
                    b = rt * 2 + bh
                    for el in range(2):
                        nc.vector.tensor_tensor(
                            out=cc[el * 64:(el + 1) * 64, b * 192:(b + 1) * 192]
                                .rearrange("p (ch bm) -> p ch bm", ch=3, bm=64),
                            in0=cf[bh * 64:(bh + 1) * 64, :].rearrange(
                                "p (o bm) -> p o bm", o=1).broadcast_to([64, 3, 64]),
                            in1=gt_s[bh * 64:(bh + 1) * 64, 8 * rt + 1 + el:8 * rt + 6 + el:2]
                                .rearrange("p (ch o) -> p ch o", ch=3, o=1)
                                .broadcast_to([64, 3, 64]),
                            op=ALU.mult,
                        )

            # ---- per-mode channel mixing: 128-col stationaries (mode pairs),
            # rhs cols (b, par); only par-diagonal PSUM quadrants are valid.
            # Band-ordered so band k's matmuls start as soon as w[k] lands.
            pb = [psp.tile([128, 64], f32, tag="pb", name=f"pb{i}") for i in range(4)]
            ccv = cc[:, :].rearrange("p (b r) -> p b r", b=4, r=192)
            for band in range(4):
                for pr in range(8):
                    base = band * 16 + pr * 2
                    for ch in range(3):
                        nc.tensor.matmul(
                            out=pb[band][:, pr * 8:(pr + 1) * 8],
                            lhsT=wt_b[band][:, (pr * 3 + ch) * 128:(pr * 3 + ch + 1) * 128],
                            rhs=ccv[:, :, ch * 64 + base:ch * 64 + base + 2],
                            start=(ch == 0), stop=(ch == 2),
                        )

            # ---- level-4 Haar synthesis from PSUM quadrants
            # SD[:, di*64 + pr*8 + b*2 + par]: di=0 -> ll+lh, di=1 -> ll-lh
            SD = sp.tile([128, 128], f32, tag="SD", name="SD")
            TU = sp.tile([128, 128], f32, tag="TU", name="TU")
            sb1 = sp.tile([128, 64], f32, tag="sb1", name="sb1")
            sb3 = sp.tile([128, 64], f32, tag="sb3", name="sb3")
            nc.vector.tensor_copy(sb1[:, :], pb[1][:, :])
            nc.vector.tensor_copy(sb3[:, :], pb[3][:, :])
            nc.vector.tensor_add(SD[:, 0:64], pb[0][:, :], sb1[:, :])
            nc.vector.tensor_sub(SD[:, 64:128], pb[0][:, :], sb1[:, :])
            nc.vector.tensor_add(TU[:, 0:64], pb[2][:, :], sb3[:, :])
            nc.vector.tensor_sub(TU[:, 64:128], pb[2][:, :], sb3[:, :])

            # at[bh*64+o, x*16 + di*8 + yy*4 + par*2 + dj]: rt0 on Vector, rt1 on GpSimd
            adjH = []
            for rt in range(2):
                eng = nc.vector if rt == 0 else nc.gpsimd
                at = sp.tile([128, 64], f32, tag="at", name=f"at{rt}")
                for bh in range(2):
                    b = rt * 2 + bh
                    ov = at[bh * 64:(bh + 1) * 64, :].rearrange(
                        "p (x di yy par dj) -> p x di yy par dj",
                        x=4, di=2, yy=2, par=2, dj=2)
                    sv = lambda t, par: t[par * 64:(par + 1) * 64, :].rearrange(
                        "p (di x yy bb pq) -> p x di yy bb pq",
                        di=2, x=4, yy=2, bb=4, pq=2)[:, :, :, :, b, par]
                    for par in range(2):
                        eng.tensor_add(ov[:, :, :, :, par, 0], sv(SD, par), sv(TU, par))
                        eng.tensor_sub(ov[:, :, :, :, par, 1], sv(SD, par), sv(TU, par))
                # adjF = at + (-G/64 * WSCALE) * s8  (still scaled by WSCALE)
                af = sp.tile([128, 64], f32, tag="adjF", name=f"af{rt}")
                nc.vector.scalar_tensor_tensor(
                    out=af[:, :], in0=s8[rt][:, :], scalar=gt_s[:, 8 * rt + 7:8 * rt + 8],
                    in1=at[:, :], op0=ALU.mult, op1=ALU.add,
                )
                # expand over h-rep and descale: adj_h[p, u*64+dh*8+v] = adjF/WSCALE
                ah = sp.tile([128, 512], bf16, tag="adjh", name=f"ah{rt}")
                nc.vector.tensor_scalar(
                    out=ah[:, :].rearrange("p (u dh v) -> p u dh v", u=8, dh=8, v=8),
                    in0=af[:, :].rearrange("p (u o v) -> p u o v", u=8, o=1, v=8)
                        .broadcast_to([128, 8, 8, 8]),
                    scalar1=1.0 / WSCALE, scalar2=None, op0=ALU.mult,
                )
                adjH.append(ah)

            # ---- y = G*x + rep8(adj): 8 pieces across V / G / PE+ACT
            for rt in range(2):
                for p in range(4):
                    s = rt * 4 + p
                    xin = xc[rt * 2 + (1 if p == 3 else 0)][:, (1024 * p if p < 3 else 0):][:, 0:1024]
                    adj_in = adjH[rt][:, 128 * p:128 * (p + 1)] \
                        .rearrange("p (hv o) -> p hv o", o=1).broadcast_to([128, 128, 8])
                    if s in PIECE_G:
                        ys = g_ys[s]
                        nc.gpsimd.tensor_tensor(
                            out=ys[:, :].rearrange("p (hv w) -> p hv w", w=8),
                            in0=ys[:, :].rearrange("p (hv w) -> p hv w", w=8),
                            in1=adj_in, op=ALU.add,
                        )
                    elif s in PIECE_PE:
                        ys = yp.tile([128, 1024], bf16, tag="ys", name=f"y{s}")
                        py = psy.tile([128, 1024], f32, tag="py", name=f"py{s}")
                        for hh in range(2):          # moving operand max 512 cols
                            sl = slice(512 * hh, 512 * (hh + 1))
                            nc.tensor.matmul(
                                out=py[:, sl], lhsT=dg_s[:, 128 * rt:128 * (rt + 1)],
                                rhs=xin[:, sl], start=True, stop=False,
                            )
                            nc.tensor.matmul(
                                out=py[:, sl], lhsT=iw_s[:, :],
                                rhs=adjH[rt][:, 128 * p + 64 * hh:128 * p + 64 * (hh + 1)]
                                    .rearrange("p (hv o) -> p hv o", o=1)
                                    .broadcast_to([128, 64, 8]),
                                start=False, stop=True,
                            )
                        nc.scalar.copy(out=ys[:, :], in_=py[:, :])
                    else:
                        ys = yp.tile([128, 1024], bf16, tag="ys", name=f"y{s}")
                        nc.vector.scalar_tensor_tensor(
                            out=ys[:, :].rearrange("p (hv w) -> p hv w", w=8),
                            in0=xin.rearrange("p (hv w) -> p hv w", w=8),
                            scalar=gt_s[:, 8 * rt:8 * rt + 1],
                            in1=adj_in, op0=ALU.mult, op1=ALU.add,
                        )
                    nc.sync.dma_start(out=yw[rt, :, 1024 * p:1024 * (p + 1)], in_=ys[:, :])
    nc.compile()
    return nc


_NC = None


def _get_nc():
    global _NC
    if _NC is None:
        _NC = _build_nc()
    return _NC


def _pack_weights(WL, WH):
    import ml_dtypes
    NE = 6
    # Wall[band, e, i, o, x, y]; band 0 = WL, bands 1..3 = WH[:, k-1]; e = ch*2+el
    Wall = np.empty((4, NE, C, C, 4, 4), np.float32)
    Wall[0] = WL[:NE]
    for k in range(3):
        Wall[k + 1] = WH[:NE, k]
    Wall *= 0.0625 * WSCALE            # idwt/rep8 scales + fp8 range scale
    W7 = Wall.reshape(4, 3, 2, C, C, 8, 2)            # band, ch, el, i, o, pr, par
    T = W7.transpose(0, 2, 3, 5, 1, 6, 4)             # band, el, i, pr, ch, par, o
    return np.ascontiguousarray(T.reshape(4, 128, 3072)).astype(ml_dtypes.float8_e4m3fn)


def _pack_gates(lambda_):
    lam = lambda_.reshape(B, 8).astype(np.float32)
    G = lam.sum(1)
    geff = lam[:, :6].copy()
    geff[:, 4] += lam[:, 6]
    geff[:, 5] += lam[:, 7]
    gt = np.zeros((B, 8), np.float32)
    gt[:, 0] = G
    gt[:, 1:7] = geff
    gt[:, 7] = -G / 64.0 * WSCALE
    return gt


def _build_in_maps(x, lambda_, WL, WH):
    import ml_dtypes
    wtp = _pack_weights(np.asarray(WL, np.float32), np.asarray(WH, np.float32))
    gtp = _pack_gates(np.asarray(lambda_, np.float32))
    xb = np.asarray(x, np.float32).astype(ml_dtypes.bfloat16)
    iw = np.ascontiguousarray(np.eye(128, dtype=np.float32)).astype(ml_dtypes.bfloat16)

    in_maps = []
    for k in range(N_CORES):
        xl = np.ascontiguousarray(xb[k * BL:(k + 1) * BL].reshape(2, 128, 4096))
        g4 = gtp[k * BL:(k + 1) * BL]                 # [4, 8], b = rt*2+bh
        gl = np.broadcast_to(
            g4.reshape(2, 2, 1, 8).transpose(1, 2, 0, 3), (2, 64, 2, 8))
        gl = np.ascontiguousarray(gl.reshape(128, 16), dtype=np.float32)
        # dg[rt] = diag over partitions (bh,c) with value G[rt*2+bh]
        Gv = g4[:, 0].reshape(2, 2)                   # [rt, bh]
        dgl = np.zeros((2, 128, 128), np.float32)
        idx = np.arange(128)
        for rt in range(2):
            dgl[rt, idx, idx] = np.repeat(Gv[rt], 64)
        in_maps.append({"xw": xl, "wt": wtp, "gt": gl,
                        "dg": dgl.astype(ml_dtypes.bfloat16),
                        "iw": iw})
    return in_maps


def kernel(x, lambda_, WL, WH):
    from concourse.bass_utils import run_bass_kernel_spmd

    nc = _get_nc()
    in_maps = _build_in_maps(x, lambda_, WL, WH)
    res = run_bass_kernel_spmd(nc, in_maps, list(range(N_CORES)))
    out = np.empty((B, C, S, S), np.float32)
    for k in range(N_CORES):
        out[k * BL:(k + 1) * BL] = (
            res.results[k]["yw"].astype(np.float32).reshape(BL, C, S, S))
    return out
